# revision 20
# baseline (speedup 1.0000x reference)
"""Multi-head attention (B=4, S=2048, D=1024, H=16) on 8 Trainium2 cores.

Sharding: core c -> (batch b = c//2, head-half hh = c%2).  Each core computes
8 heads of one batch: QKV projections with column-sliced weights, attention,
and a partial output projection with row-sliced Wo.  Host sums the two
partial outputs per batch and adds the constant bias terms.

All matmuls run as float32r (full PE rate); softmax exp on the scalar
engine.  Attention is computed in "transposed" orientation (features/keys on
partitions) so no on-chip transposes are needed:
  - Q^T, K^T: [512 feat, 2048 tok] via lhsT=W chunk, rhs=X^T chunk
  - V natural: [2048 tok, 512 feat] via lhsT=X^T chunk, rhs=Wv chunk
  - S^T[k,q]: lhsT=K^T[64,kblk], rhs=Q^T[64,qchunk] (two heads row-packed)
  - P^T = exp(S^T) on ACT, PSUM->SBUF
  - PV: out^T[dk,q] accum over kblk: lhsT=V[kblk, dk], rhs=P^T (two heads
    col-packed); Z row sums via lhsT=ones col-packed alongside
  - normalize x^T by 1/Z via a K=2 outer-product broadcast matmul + DVE mul
  - y[q, fout] natural: lhsT=x^T slice, rhs=Wo rows
"""
import numpy as np

import concourse.tile as tile
from concourse import bacc, mybir
from concourse import bass_utils

F32 = mybir.dt.float32
F32R = mybir.dt.float32r
BF16 = mybir.dt.bfloat16
EXP = mybir.ActivationFunctionType.Exp

B, S, D = 4, 2048, 1024
H = 16
DK = 64
FEAT = 512          # features per core (8 heads)
N_CORES = 8

_PROGRAM = None


def _build_program():
    nc = bacc.Bacc("TRN2", target_bir_lowering=False, debug=False,
                   enable_asserts=True, num_devices=N_CORES)

    xq = nc.dram_tensor("xq_t", [D, S], F32, kind="ExternalInput").ap()
    xk = nc.dram_tensor("xk_t", [D, S], F32, kind="ExternalInput").ap()
    xv = nc.dram_tensor("xv_t", [D, S], F32, kind="ExternalInput").ap()
    wq = nc.dram_tensor("wq", [D, FEAT], F32, kind="ExternalInput").ap()
    wk = nc.dram_tensor("wk", [D, FEAT], F32, kind="ExternalInput").ap()
    wv = nc.dram_tensor("wv", [D, FEAT], F32, kind="ExternalInput").ap()
    wo = nc.dram_tensor("wo", [FEAT, D], F32, kind="ExternalInput").ap()
    bq = nc.dram_tensor("bq", [FEAT, 1], F32, kind="ExternalInput").ap()
    bk = nc.dram_tensor("bk", [FEAT, 1], F32, kind="ExternalInput").ap()
    sel = nc.dram_tensor("sel", [2, 128], F32, kind="ExternalInput").ap()
    ones = nc.dram_tensor("ones", [128, 64], F32, kind="ExternalInput").ap()
    y = nc.dram_tensor("y", [S, D], F32, kind="ExternalOutput").ap()

    with tile.TileContext(nc) as tc:
        with nc.allow_low_precision(reason="fp32r matmul operand tiles"):
            _emit(nc, tc, xq, xk, xv, wq, wk, wv, wo, bq, bk, sel, ones, y)
    nc.compile()
    return nc


def _emit(nc, tc, xq, xk, xv, wq, wk, wv, wo, bq, bk, sel, ones, y):
    from contextlib import ExitStack

    MM = nc.tensor.matmul

    with ExitStack() as ctx:
        ep = ctx.enter_context

        # ---------- persistent SBUF ----------
        qt_pool = ep(tc.tile_pool(name="qt", bufs=1))
        kt_pool = ep(tc.tile_pool(name="kt", bufs=1))
        v_pool = ep(tc.tile_pool(name="v", bufs=1))
        misc_pool = ep(tc.tile_pool(name="misc", bufs=1))

        qt = [qt_pool.tile([128, S], F32R, tag=f"qt{m}", name=f"qt{m}") for m in range(4)]
        kt = [kt_pool.tile([128, S], F32R, tag=f"kt{m}", name=f"kt{m}") for m in range(4)]
        v_sb = [v_pool.tile([128, FEAT], BF16, tag=f"v{k}", name=f"v{k}") for k in range(16)]

        bq_sb = misc_pool.tile([128, 4], F32, tag="bq")
        bk_sb = misc_pool.tile([128, 4], F32, tag="bk")
        ones_sb = misc_pool.tile([128, 64], BF16, tag="ones")
        selA_sb = misc_pool.tile([1, 128], F32R, tag="selA")
        selB_sb = misc_pool.tile([1, 128], F32R, tag="selB")

        # ---------- projections ----------
        with tc.tile_pool(name="xin", bufs=2) as xin_pool, \
             tc.tile_pool(name="w", bufs=2) as w_pool:

            # V natural: [2048, 512] in 16 row blocks
            xv_sb = [xin_pool.tile([128, S], BF16, tag=f"x{c}", name=f"x{c}") for c in range(8)]
            wv_sb = [w_pool.tile([128, FEAT], BF16, tag=f"w{c}", name=f"w{c}") for c in range(8)]
            for c in range(8):
                nc.gpsimd.dma_start(xv_sb[c][:], xv[c * 128:(c + 1) * 128, :])
                nc.gpsimd.dma_start(wv_sb[c][:], wv[c * 128:(c + 1) * 128, :])
            # small parameter loads issued after the bulk V input loads
            for m in range(4):
                nc.sync.dma_start(bq_sb[:, m:m + 1], bq[m * 128:(m + 1) * 128, 0:1])
                nc.sync.dma_start(bk_sb[:, m:m + 1], bk[m * 128:(m + 1) * 128, 0:1])
            nc.gpsimd.dma_start(ones_sb[:], ones)
            nc.sync.dma_start(selA_sb[:], sel[0:1, :].bitcast(F32R))
            nc.sync.dma_start(selB_sb[:], sel[1:2, :].bitcast(F32R))
            with tc.tile_pool(name="vps", bufs=1, space="PSUM") as vps_pool:
                for khalf in range(2):
                    vps = [vps_pool.tile([128, FEAT], F32, tag=f"vps{j}", name=f"vps{j}")
                           for j in range(8)]
                    for c in range(8):
                        for j in range(8):
                            kb = khalf * 8 + j
                            MM(vps[j][:],
                               xv_sb[c][:, kb * 128:(kb + 1) * 128],
                               wv_sb[c][:],
                               start=(c == 0), stop=(c == 7))
                    for j in range(8):
                        nc.vector.tensor_copy(v_sb[khalf * 8 + j][:], vps[j][:])

            # K^T then Q^T: [512, 2048] in 4 feature blocks
            with tc.tile_pool(name="pps", bufs=2, space="PSUM") as pj_pool:
                for name, xdram, wdram, dst, bias_sb in (
                        ("k", xk, wk, kt, bk_sb),
                        ("q", xq, wq, qt, bq_sb)):
                    x_sb = [xin_pool.tile([128, S], BF16, tag=f"x{c}", name=f"x{name}{c}")
                            for c in range(8)]
                    w_sb = [w_pool.tile([128, FEAT], BF16, tag=f"w{c}", name=f"w{name}{c}")
                            for c in range(8)]
                    for c in range(8):
                        nc.gpsimd.dma_start(
                            x_sb[c][:], xdram[c * 128:(c + 1) * 128, :])
                        nc.gpsimd.dma_start(
                            w_sb[c][:], wdram[c * 128:(c + 1) * 128, :])
                    for m in range(4):
                        ps = pj_pool.tile([128, S], F32, tag="pj")
                        for c in range(8):
                            for n in range(4):
                                MM(ps[:, n * 512:(n + 1) * 512],
                                   w_sb[c][:, m * 128:(m + 1) * 128],
                                   x_sb[c][:, n * 512:(n + 1) * 512],
                                   start=(c == 0), stop=(c == 7))
                        for n in range(4):
                            nc.vector.tensor_scalar_add(
                                dst[m][:, n * 512:(n + 1) * 512],
                                ps[:, n * 512:(n + 1) * 512],
                                bias_sb[:, m:m + 1])

        # ---------- attention ----------
        wo_pool = ep(tc.tile_pool(name="wo", bufs=1))
        xT_pool = ep(tc.tile_pool(name="xT", bufs=1))
        pt_pool = ep(tc.tile_pool(name="pt", bufs=4))
        rz_pool = ep(tc.tile_pool(name="rz", bufs=2))
        y_sb_pool = ep(tc.tile_pool(name="ysb", bufs=4))

        wo_sb = [wo_pool.tile([128, D], F32R, tag=f"wo{p}", name=f"wo{p}") for p in range(4)]
        for p in range(4):
            nc.sync.dma_start(wo_sb[p][:], wo[p * 128:(p + 1) * 128, :].bitcast(F32R))
        xT = [xT_pool.tile([128, S], F32R, tag=f"xT{p}", name=f"xT{p}") for p in range(4)]

        with tc.tile_pool(name="st", bufs=2, space="PSUM") as st_pool, \
             tc.tile_pool(name="pv", bufs=2, space="PSUM") as pv_pool, \
             tc.tile_pool(name="zp", bufs=2, space="PSUM") as zp_pool:
            warm = st_pool.tile([128, 1024], F32, tag="st", name="warm")
            for i in range(20):
                MM(warm[:, 0:512], kt[0][:, 0:128], qt[0][:, 0:512],
                   start=True, stop=True)
            pend_recip = []
            pend_norm = []
            pend_yproj = []

            def flush_yproj_unit():
                if not pend_yproj:
                    return
                qb, fo = pend_yproj.pop(0)
                yp = zp_pool.tile([128, 512], F32, tag="zp", name="yp")
                for pp in range(4):
                    MM(yp[:],
                       xT[pp][:, qb * 128:(qb + 1) * 128],
                       wo_sb[pp][:, fo * 512:(fo + 1) * 512],
                       start=(pp == 0), stop=(pp == 3))
                ysb = y_sb_pool.tile([128, 512], F32, tag="ysb")
                nc.vector.tensor_copy(ysb[:], yp[:])
                nc.sync.dma_start(
                    y[qb * 128:(qb + 1) * 128, fo * 512:(fo + 1) * 512], ysb[:])

            def flush_recips():
                for (fp, fqo, fzA, fzB) in pend_recip:
                    rzA = rz_pool.tile([1, 512], F32R, tag="rzA", name="rzA")
                    rzB = rz_pool.tile([1, 512], F32R, tag="rzB", name="rzB")
                    nc.vector.reciprocal(rzA[:], fzA[:])
                    nc.vector.reciprocal(rzB[:], fzB[:])
                    pend_norm.append((fp, fqo, rzA, rzB))
                pend_recip.clear()

            def flush_norms():
                for (fp, fqo, frzA, frzB) in pend_norm:
                    bc = pv_pool.tile([128, 512], F32, tag="pv", name="bc")
                    MM(bc[:], selA_sb[:], frzA[:], start=True, stop=False)
                    MM(bc[:], selB_sb[:], frzB[:], start=False, stop=True)
                    nc.vector.tensor_mul(xT[fp][:, fqo:fqo + 512],
                                         xT[fp][:, fqo:fqo + 512], bc[:])
                pend_norm.clear()

            for qq in range(4):
                for p in range(4):
                    qo = qq * 512
                    pv = pv_pool.tile([128, 512], F32, tag="pv")
                    zp = zp_pool.tile([128, 512], F32, tag="zp")

                    def emit_pvz(pt, kb):
                        # PV accumulation + Z row sums, two heads col-packed
                        MM(pv[0:64, :],
                           v_sb[kb][:, p * 128:p * 128 + 64],
                           pt[:, 0:512],
                           tile_position=(0, 0),
                           start=(kb == 0), stop=(kb == 15))
                        MM(pv[64:128, :],
                           v_sb[kb][:, p * 128 + 64:p * 128 + 128],
                           pt[:, 512:1024],
                           tile_position=(0, 64),
                           start=(kb == 0), stop=(kb == 15),
                           skip_group_check=True)
                        MM(zp[0:64, :],
                           ones_sb[:],
                           pt[:, 0:512],
                           tile_position=(0, 0),
                           start=(kb == 0), stop=(kb == 15))
                        MM(zp[64:128, :],
                           ones_sb[:],
                           pt[:, 512:1024],
                           tile_position=(0, 64),
                           start=(kb == 0), stop=(kb == 15),
                           skip_group_check=True)

                    prev = None
                    for kb in range(16):
                        if kb == 2:
                            flush_recips()
                        if kb == 12:
                            flush_norms()
                        if p in (1, 2) and kb in (3, 5, 7, 9):
                            flush_yproj_unit()
                        ko = kb * 128
                        st = st_pool.tile([128, 1024], F32, tag="st")
                        # scores^T, two heads row-packed (K=64 each)
                        MM(st[:, 0:512],
                           kt[p][0:64, ko:ko + 128],
                           qt[p][0:64, qo:qo + 512],
                           start=True, stop=True)
                        MM(st[:, 512:1024],
                           kt[p][64:128, ko:ko + 128],
                           qt[p][64:128, qo:qo + 512],
                           start=True, stop=True)
                        pt = pt_pool.tile([128, 1024], BF16, tag="pt")
                        nc.scalar.activation(pt[:], st[:], EXP)
                        if prev is not None:
                            emit_pvz(*prev)
                        prev = (pt, kb)
                    emit_pvz(*prev)
                    zA = rz_pool.tile([1, 512], F32, tag="zA", name="zA", bufs=3)
                    zB = rz_pool.tile([1, 512], F32, tag="zB", name="zB", bufs=3)
                    nc.vector.tensor_copy(zA[:], zp[0:1, :])
                    nc.vector.tensor_copy(zB[:], zp[64:65, :])
                    nc.vector.tensor_copy(xT[p][:, qo:qo + 512], pv[:])
                    pend_recip.append((p, qo, zA, zB))
                for qb in range(4 * qq, 4 * qq + 4):
                    for fo in range(2):
                        pend_yproj.append((qb, fo))
            flush_recips()
            flush_norms()
            while pend_yproj:
                flush_yproj_unit()



def get_program():
    global _PROGRAM
    if _PROGRAM is None:
        _PROGRAM = _build_program()
    return _PROGRAM


def make_in_maps(Q_in, K_in, V_in, Wq, bq, Wk, bk, Wv, bv, Wo, bo):
    """Shard full inputs into 8 per-core input maps."""
    scale = np.float32(1.0 / np.sqrt(DK))
    sel = np.zeros((2, 128), np.float32)
    sel[0, 0:64] = 1.0
    sel[1, 64:128] = 1.0
    ones = np.ones((128, 64), np.float32)

    xt = {}
    for b in range(B):
        xt[b] = (np.ascontiguousarray(np.asarray(Q_in[b], np.float32).T),
                 np.ascontiguousarray(np.asarray(K_in[b], np.float32).T),
                 np.ascontiguousarray(np.asarray(V_in[b], np.float32).T))

    in_maps = []
    for c in range(N_CORES):
        b, hh = c // 2, c % 2
        sl = slice(hh * FEAT, (hh + 1) * FEAT)
        in_maps.append({
            "xq_t": xt[b][0],
            "xk_t": xt[b][1],
            "xv_t": xt[b][2],
            "wq": np.ascontiguousarray(np.asarray(Wq, np.float32)[:, sl]),
            "wk": np.ascontiguousarray(np.asarray(Wk, np.float32)[:, sl] * scale),
            "wv": np.ascontiguousarray(np.asarray(Wv, np.float32)[:, sl]),
            "wo": np.ascontiguousarray(np.asarray(Wo, np.float32)[sl, :]),
            "bq": np.ascontiguousarray(np.asarray(bq, np.float32)[sl, None]),
            "bk": np.ascontiguousarray(np.asarray(bk, np.float32)[sl, None] * scale),
            "sel": sel,
            "ones": ones,
        })
    return in_maps


def gather_output(results, Wo, bv, bo):
    """Combine per-core partial outputs into the full [B, S, D] output."""
    const = (np.asarray(bv, np.float32) @ np.asarray(Wo, np.float32)
             + np.asarray(bo, np.float32))
    out = np.empty((B, S, D), np.float32)
    for b in range(B):
        out[b] = results[2 * b]["y"] + results[2 * b + 1]["y"] + const
    return out


def kernel(Q_in, K_in, V_in, Wq, bq, Wk, bk, Wv, bv, Wo, bo):
    nc = get_program()
    in_maps = make_in_maps(Q_in, K_in, V_in, Wq, bq, Wk, bk, Wv, bv, Wo, bo)
    res = bass_utils.run_bass_kernel_spmd(nc, in_maps, core_ids=list(range(N_CORES)))
    return gather_output(res.results, Wo, bv, bo)
